# revision 19
# baseline (speedup 1.0000x reference)
"""Edge-parallel GNN message-passing kernel for TRN2 (8 NeuronCores).

Reference computation (DTIConvGraph3):
    hs = atom_feats[src]; hd = atom_feats[dst]
    init = concat([hs, hd, bond], 1)
    pre  = init @ W1.T + b1 + (hs+hd) @ W2.T + b2
    out  = leaky_relu(pre, 0.01)

Algebraic restructuring: with W1 = [W1s | W1d | w1b] (columns 0:128,
128:256, 256) the per-edge matmuls collapse to per-node ones:
    As = atom @ (W1s + W2).T            # [N, 128]
    Ad = atom @ (W1d + W2).T            # [N, 128]
    pre[e] = As[src[e]] + Ad[dst[e]] + bond[e]*w1b + (b1 + b2)

Phase 1 (on device, tiny): compute AsAd = [As | Ad] into an HBM table.
Phase 2 (memory-bound): per-edge dma_gather of 512B rows from the table,
DVE fused ops for the bond term and leaky_relu, contiguous DMA out.

Sharding: edges split evenly across the 8 cores; node table + weights
replicated (classic edge-parallel GNN sharding).
"""

import os
import sys

import ml_dtypes
import numpy as np

if "/opt/trn_rl_repo" not in sys.path:
    sys.path.insert(0, "/opt/trn_rl_repo")

import concourse.bacc as bacc
import concourse.bass as bass
import concourse.mybir as mybir
from concourse.bass_utils import run_bass_kernel_spmd
from concourse.tile import TileContext

BF16 = ml_dtypes.bfloat16

# Problem dims (hardcoded per contest rules: kernel.py is self-contained)
N = 10000            # nodes
D = 128              # node feature dim == out dim
E = 320000           # edges
N_CORES = 8
EC = E // N_CORES    # 40000 edges per core

# Tiling
TILE_E = 2048        # edges per phase-2 tile
NBLK = TILE_E // 128  # 16
NT = (EC + TILE_E - 1) // TILE_E  # 20
ECP = NT * TILE_E    # 40960 padded edges per core
NPAD = ((N + 127) // 128) * 128   # 10112 padded nodes

NEG_SLOPE = 0.01

# Set by test harness to capture a profile; kernel() stores timing here.
KERNEL_TRACE = False
LAST_EXEC_NS = None
LAST_RESULTS = None

_PROGRAM = None


def _build_program(
    npad=NPAD,
    tile_e=TILE_E,
    nt=NT,
    gather_bufs=4,
    acc_bufs=4,
    p1_bufs=4,
    repeat=1,
    repeat_all=1,  # loop const-loads+phase1+phase2 (timing the full body)
    single_packet=False,  # unpacketed: allows 2048-idx gathers (sp=1 caps at 1024)
    gather_chunk=2048,  # one gather per side per tile
    gather_queues=4,    # spread gathers over all 4 SWDGE queues (parallel Q7 gen)
    variant=5,  # 5=broadcast-AP bond + ACT lrelu (fastest); 0=16-stt bond
    # other variants: 1=no gathers (memset), 2=src only, 3/4=ACT bond, 6=DVE lrelu
):
    single_packet = bool(single_packet)
    f32 = mybir.dt.float32
    bf16 = mybir.dt.bfloat16
    i16 = mybir.dt.int16
    nblk = tile_e // 128
    ecp = nt * tile_e
    s16 = tile_e // 16  # idx columns per tile

    nc = bacc.Bacc(
        "TRN2",
        target_bir_lowering=False,
        debug=False,
        num_devices=N_CORES,
        num_swdge_queues=gather_queues,
    )
    atomT = nc.declare_dram_parameter("atomT", [128, npad], bf16, False)
    wswdT = nc.declare_dram_parameter("wswdT", [128, 256], bf16, False)
    bias2 = nc.declare_dram_parameter("bias2", [1, 256], bf16, False)
    w1bbc = nc.declare_dram_parameter("w1bbc", [128, 128], bf16, False)
    sidx = nc.declare_dram_parameter("sidx", [128, ecp // 16], i16, False)
    didx = nc.declare_dram_parameter("didx", [128, ecp // 16], i16, False)
    # bond pre-permuted on host: bond[p, t*nblk+b] = bond_feats[t*tile_e + p*nblk + b]
    bond = nc.declare_dram_parameter("bond", [128, ecp // 128], f32, False)
    # out[t, p, b*128+f] = result row (t*tile_e + p*nblk + b), feature f —
    # flattening [nt, 128, nblk] row-major IS natural edge order.
    out = nc.declare_dram_parameter("out", [nt, 128, nblk * 128], bf16, True)
    asad = nc.dram_tensor("asad", [npad, 256], bf16)

    mult = mybir.AluOpType.mult
    add = mybir.AluOpType.add
    amax = mybir.AluOpType.max

    import contextlib

    with TileContext(nc) as tc:
        with (
            tc.tile_pool(name="const", bufs=1) as const,
            tc.tile_pool(name="psum", bufs=p1_bufs, space="PSUM") as psum,
            tc.tile_pool(name="p1", bufs=p1_bufs) as p1,
            tc.tile_pool(name="g", bufs=gather_bufs) as g,
            tc.tile_pool(name="acc", bufs=acc_bufs) as acc,
            tc.For_i(0, repeat_all, 1) if repeat_all > 1 else contextlib.nullcontext(),
        ):
            atomT_sb = const.tile([128, npad], bf16)
            nc.sync.dma_start(atomT_sb[:], atomT[:])
            w_sb = const.tile([128, 256], bf16)
            nc.sync.dma_start(w_sb[:], wswdT[:])
            b_sb = const.tile([1, 256], bf16)
            nc.sync.dma_start(b_sb[:], bias2[:])
            w1b_sb = const.tile([128, 128], bf16)
            nc.sync.dma_start(w1b_sb[:], w1bbc[:])
            sidx_sb = const.tile([128, ecp // 16], i16)
            nc.sync.dma_start(sidx_sb[:], sidx[:])
            didx_sb = const.tile([128, ecp // 16], i16)
            nc.sync.dma_start(didx_sb[:], didx[:])
            bond_sb = const.tile([128, ecp // 128], f32)
            nc.sync.dma_start(bond_sb[:], bond[:])
            ones_sb = const.tile([1, 128], bf16)
            nc.vector.memset(ones_sb[:], 1.0)

            # ---- Phase 1: AsAd[n, 0:128]=As, [128:256]=Ad (+bias on As half)
            for i in range(npad // 128):
                ps = psum.tile([128, 256], f32, tag="ps")
                nc.tensor.matmul(
                    ps[:],
                    atomT_sb[:, i * 128 : (i + 1) * 128],
                    w_sb[:],
                    start=True,
                    stop=False,
                )
                nc.tensor.matmul(ps[:], ones_sb[:], b_sb[:], start=False, stop=True)
                ab = p1.tile([128, 256], bf16, tag="ab")
                nc.scalar.copy(ab[:], ps[:])
                nc.sync.dma_start(asad[i * 128 : (i + 1) * 128, :], ab[:])

            tc.strict_bb_all_engine_barrier()

            # ---- Phase 2: per-edge gather + fused elementwise
            # (repeat>1 re-runs phase 2 in a device-side loop writing
            # identical values — used only by the wall-clock differencing
            # timer. The body never references the induction variable.)
            loop_cm = (
                tc.For_i(0, repeat, 1)
                if repeat > 1
                else contextlib.nullcontext()
            )
            with loop_cm:
                phase2(
                    nc, tc, nt, tile_e, nblk, s16, gather_chunk, single_packet,
                    gather_queues, g, acc, f32, asad, sidx_sb, didx_sb,
                    bond_sb, w1b_sb, out, mult, add, amax, variant,
                )
    nc.compile()
    return nc


def phase2(
    nc, tc, nt, tile_e, nblk, s16, gather_chunk, single_packet, gather_queues,
    g, acc, f32, asad, sidx_sb, didx_sb, bond_sb, w1b_sb, out, mult, add, amax,
    variant=0,
):
    bf16 = mybir.dt.bfloat16
    ck = min(gather_chunk or tile_e, tile_e)
    assert tile_e % ck == 0 and ck % 128 == 0
    # One shared register for the (constant) gather index count — a fresh
    # to_reg per gather costs a Pool-SEQ RegisterMove (~1us each, ~80/iter).
    nidx_reg = nc.gpsimd.to_reg(ck)
    for t in range(nt):
        ts_ = g.tile([128, nblk, 128], bf16, tag="ts")
        td_ = g.tile([128, nblk, 128], bf16, tag="td")
        gq = 0
        if variant == 1:
            nc.vector.memset(ts_[:], 0.25)
        if variant in (1, 2):
            nc.vector.memset(td_[:], 0.25)
        for c0 in range(0, tile_e, ck) if variant != 1 else []:
            # chunk c0..c0+ck of slot space = output blocks
            # c0//128..(c0+ck)//128, idx columns c0//16..(c0+ck)//16
            for tile_, idxs_sb, col0 in (
                (ts_, sidx_sb, 0),
                (td_, didx_sb, 128),
            )[: (1 if variant == 2 else 2)]:
                nc.gpsimd.dma_gather(
                    tile_[:, c0 // 128 : (c0 + ck) // 128, :],
                    asad[:, col0 : col0 + 128],
                    idxs_sb[
                        :, t * s16 + c0 // 16 : t * s16 + (c0 + ck) // 16
                    ],
                    ck,
                    nidx_reg,
                    128,
                    elem_step=256,
                    single_packet=single_packet,
                    queue_num=(t * (tile_e // ck) * 2 + gq) % gather_queues,
                )
                gq += 1
        pre = acc.tile([128, nblk, 128], bf16, tag="pre")
        if variant in (5, 6):
            # bond term in one broadcast DVE op: bt[p,b,f] = w1b[f]*bond[p,b]
            bt = acc.tile([128, nblk, 128], bf16, tag="bt")
            nc.vector.tensor_tensor(
                bt[:],
                w1b_sb[:].rearrange("p (b f) -> p b f", b=1).to_broadcast(
                    [128, nblk, 128]
                ),
                bond_sb[:, t * nblk : (t + 1) * nblk].to_broadcast(
                    [128, nblk, 128]
                ),
                mult,
            )
            nc.vector.tensor_add(pre[:], ts_[:], td_[:])
            nc.vector.tensor_add(pre[:], pre[:], bt[:])
            ob = acc.tile([128, nblk, 128], bf16, tag="ob")
            if variant == 5:
                nc.scalar.activation(
                    ob[:], pre[:], mybir.ActivationFunctionType.Lrelu,
                    alpha=NEG_SLOPE,
                )
            else:
                nc.vector.scalar_tensor_tensor(
                    ob[:], pre[:], NEG_SLOPE, pre[:], op0=mult, op1=amax
                )
            nc.sync.dma_start(
                out[t, :, :], ob[:, :, :].rearrange("p b f -> p (b f)")
            )
            continue
        nc.vector.tensor_add(pre[:], ts_[:], td_[:])
        if variant in (3, 4):
            # bond term on the (otherwise idle) scalar engine:
            # bt[:, b, :] = Copy(w1b * bond_scalar), then one DVE add.
            bt = acc.tile([128, nblk, 128], bf16, tag="bt")
            for b in range(nblk):
                nc.scalar.activation(
                    bt[:, b, :],
                    w1b_sb[:],
                    mybir.ActivationFunctionType.Copy,
                    scale=bond_sb[:, t * nblk + b : t * nblk + b + 1],
                )
            nc.vector.tensor_add(pre[:], pre[:], bt[:])
        else:
            for b in range(nblk):
                # pre[:, b, :] += w1b * bond  (bond scalar per partition)
                nc.vector.scalar_tensor_tensor(
                    pre[:, b, :],
                    w1b_sb[:],
                    bond_sb[:, t * nblk + b : t * nblk + b + 1],
                    pre[:, b, :],
                    op0=mult,
                    op1=add,
                )
        ob = acc.tile([128, nblk, 128], bf16, tag="ob")
        # leaky_relu: max(x, 0.01*x)
        if variant == 4:
            nc.scalar.activation(
                ob[:], pre[:], mybir.ActivationFunctionType.Lrelu,
                alpha=NEG_SLOPE,
            )
        else:
            nc.vector.scalar_tensor_tensor(
                ob[:], pre[:], NEG_SLOPE, pre[:], op0=mult, op1=amax
            )
        nc.sync.dma_start(out[t, :, :], ob[:, :, :].rearrange("p b f -> p (b f)"))


def _get_program():
    global _PROGRAM
    if _PROGRAM is None:
        _PROGRAM = _build_program()
    return _PROGRAM


def _wrap_idx(vals, tile_e=TILE_E, nt=NT):
    """[ecp] int array -> [128, ecp//16] int16 in dma_gather layout.

    Slot i of tile t gathers the row for edge t*tile_e + (i%128)*nblk +
    (i//128) (so the output tile DMAs contiguously per partition), and
    slot i's index lives at partition i%16 (replicated x8), column i//16.
    """
    nblk = tile_e // 128
    i = np.arange(tile_e)
    perm = (i % 128) * nblk + (i // 128)
    lst = vals.reshape(nt, tile_e)[:, perm]            # slot order per tile
    w = lst.reshape(nt, tile_e // 16, 16).transpose(0, 2, 1)  # [nt, 16, s16]
    w = np.tile(w, (1, 8, 1))                          # replicate to 128 parts
    return np.ascontiguousarray(
        w.transpose(1, 0, 2).reshape(128, -1)
    ).astype(np.int16)


def _host_prep(inputs):
    atom = np.ascontiguousarray(np.asarray(inputs["atom_feats"], dtype=np.float32))
    bondf = np.asarray(inputs["bond_feats"], dtype=np.float32).reshape(-1)
    src = np.asarray(inputs["src"]).astype(np.int64)
    dst = np.asarray(inputs["dst"]).astype(np.int64)
    W1 = np.asarray(inputs["W1"], dtype=np.float32)
    b1 = np.asarray(inputs["b1"], dtype=np.float32)
    W2 = np.asarray(inputs["W2"], dtype=np.float32)
    b2 = np.asarray(inputs["b2"], dtype=np.float32)

    Ws = W1[:, :D] + W2                     # [128, 128]
    Wd = W1[:, D : 2 * D] + W2              # [128, 128]
    w1b = W1[:, 2 * D]                      # [128]
    bias = b1 + b2                          # [128]

    atomT = np.zeros((128, NPAD), BF16)
    atomT[:, :N] = atom.T.astype(BF16)
    wswdT = np.ascontiguousarray(
        np.concatenate([Ws.T, Wd.T], axis=1).astype(BF16)
    )                                       # [128(fin), 256]
    bias2 = np.concatenate([bias, np.zeros(D, np.float32)])[None, :]
    bias2 = np.ascontiguousarray(bias2.astype(BF16))
    w1bbc = np.ascontiguousarray(np.tile(w1b[None, :], (128, 1)).astype(BF16))

    in_maps = []
    for c in range(N_CORES):
        sl = slice(c * EC, (c + 1) * EC)
        sp = np.zeros(ECP, np.int64)
        sp[:EC] = src[sl]
        dp = np.zeros(ECP, np.int64)
        dp[:EC] = dst[sl]
        bp = np.zeros(ECP, np.float32)
        bp[:EC] = bondf[sl]
        # bond[p, t*NBLK+b] = bp[t*TILE_E + p*NBLK + b]
        bperm = np.ascontiguousarray(
            bp.reshape(NT, 128, NBLK).transpose(1, 0, 2).reshape(128, NT * NBLK)
        )
        in_maps.append(
            {
                "atomT": atomT,
                "wswdT": wswdT,
                "bias2": bias2,
                "w1bbc": w1bbc,
                "sidx": _wrap_idx(sp),
                "didx": _wrap_idx(dp),
                "bond": bperm,
            }
        )
    return in_maps


def kernel(**inputs) -> np.ndarray:
    global LAST_EXEC_NS, LAST_RESULTS
    in_maps = _host_prep(inputs)
    nc = _get_program()
    res = run_bass_kernel_spmd(
        nc, in_maps, list(range(N_CORES)), trace=KERNEL_TRACE
    )
    LAST_EXEC_NS = res.exec_time_ns
    LAST_RESULTS = res
    outs = [
        np.asarray(res.results[c]["out"]).reshape(ECP, 128)[:EC]
        for c in range(N_CORES)
    ]
    return np.concatenate(outs, axis=0).astype(np.float32)



# revision 29
# speedup vs baseline: 1.0886x; 1.0886x over previous
"""Edge-parallel GNN message-passing kernel for TRN2 (8 NeuronCores).

Reference computation (DTIConvGraph3):
    hs = atom_feats[src]; hd = atom_feats[dst]
    init = concat([hs, hd, bond], 1)
    pre  = init @ W1.T + b1 + (hs+hd) @ W2.T + b2
    out  = leaky_relu(pre, 0.01)

Algebraic restructuring: with W1 = [W1s | W1d | w1b] (columns 0:128,
128:256, 256) the per-edge matmuls collapse to per-node ones:
    As = atom @ (W1s + W2).T            # [N, 128]
    Ad = atom @ (W1d + W2).T            # [N, 128]
    pre[e] = As[src[e]] + Ad[dst[e]] + bond[e]*w1b + (b1 + b2)

Phase 1 (on device, tiny): compute AsAd = [As | Ad] into an HBM table.
Phase 2 (memory-bound): per-edge dma_gather of 512B rows from the table,
DVE fused ops for the bond term and leaky_relu, contiguous DMA out.

Sharding: edges split evenly across the 8 cores; node table + weights
replicated (classic edge-parallel GNN sharding).
"""

import os
import sys

import ml_dtypes
import numpy as np

if "/opt/trn_rl_repo" not in sys.path:
    sys.path.insert(0, "/opt/trn_rl_repo")

import concourse.bacc as bacc
import concourse.bass as bass
import concourse.mybir as mybir
from concourse.bass_utils import run_bass_kernel_spmd
from concourse.tile import TileContext

BF16 = ml_dtypes.bfloat16

# Problem dims (hardcoded per contest rules: kernel.py is self-contained)
N = 10000            # nodes
D = 128              # node feature dim == out dim
E = 320000           # edges
N_CORES = 8
EC = E // N_CORES    # 40000 edges per core

# Tiling
TILE_E = 2048        # edges per phase-2 tile
NBLK = TILE_E // 128  # 16
NT = (EC + TILE_E - 1) // TILE_E  # 20
ECP = NT * TILE_E    # 40960 padded edges per core
NPAD = ((N + 127) // 128) * 128   # 10112 padded nodes

NEG_SLOPE = 0.01

# Set by test harness to capture a profile; kernel() stores timing here.
KERNEL_TRACE = False
LAST_EXEC_NS = None
LAST_RESULTS = None

_PROGRAM = None


def _build_program(
    npad=NPAD,
    tile_e=TILE_E,
    nt=NT,
    gather_bufs=4,
    acc_bufs=4,
    p1_bufs=4,
    repeat=1,
    repeat_all=1,  # loop const-loads+phase1+phase2 (timing the full body)
    single_packet=False,  # unpacketed: allows 2048-idx gathers (sp=1 caps at 1024)
    gather_chunk=2048,  # one gather per side per tile
    gather_queues=4,    # spread gathers over all 4 SWDGE queues (parallel Q7 gen)
    p1_batch=4,         # node-chunks per phase-1 PSUM tile
    atom_slices=1,      # >1 splits atomT load (no sim win; kept for experiments)
    variant=5,  # 5=broadcast-AP bond + ACT lrelu (fastest); 0=16-stt bond
    # other variants: 1=no gathers (memset), 2=src only, 3/4=ACT bond, 6=DVE lrelu
):
    single_packet = bool(single_packet)
    f32 = mybir.dt.float32
    bf16 = mybir.dt.bfloat16
    i16 = mybir.dt.int16
    nblk = tile_e // 128
    ecp = nt * tile_e
    s16 = tile_e // 16  # idx columns per tile

    nc = bacc.Bacc(
        "TRN2",
        target_bir_lowering=False,
        debug=False,
        num_devices=N_CORES,
        num_swdge_queues=gather_queues,
    )
    atomT = nc.declare_dram_parameter("atomT", [128, npad], bf16, False)
    wswdT = nc.declare_dram_parameter("wswdT", [128, 256], bf16, False)
    bias2 = nc.declare_dram_parameter("bias2", [1, 256], bf16, False)
    w1bbc = nc.declare_dram_parameter("w1bbc", [128, 128], bf16, False)
    sidx = nc.declare_dram_parameter("sidx", [128, ecp // 16], i16, False)
    didx = nc.declare_dram_parameter("didx", [128, ecp // 16], i16, False)
    # bond pre-permuted on host: bond[p, t*nblk+b] = bond_feats[t*tile_e + p*nblk + b]
    bond = nc.declare_dram_parameter("bond", [128, ecp // 128], f32, False)
    # out[t, p, b*128+f] = result row (t*tile_e + p*nblk + b), feature f —
    # flattening [nt, 128, nblk] row-major IS natural edge order.
    out = nc.declare_dram_parameter("out", [nt, 128, nblk * 128], bf16, True)
    asad = nc.dram_tensor("asad", [npad, 256], bf16)

    mult = mybir.AluOpType.mult
    add = mybir.AluOpType.add
    amax = mybir.AluOpType.max

    import contextlib

    with TileContext(nc) as tc:
        with (
            tc.tile_pool(name="const", bufs=1) as const,
            tc.tile_pool(name="psum", bufs=p1_bufs, space="PSUM") as psum,
            tc.tile_pool(name="p1", bufs=p1_bufs) as p1,
            tc.tile_pool(name="g", bufs=gather_bufs) as g,
            tc.tile_pool(name="acc", bufs=acc_bufs) as acc,
            tc.For_i(0, repeat_all, 1) if repeat_all > 1 else contextlib.nullcontext(),
        ):
            atomT_sb = const.tile([128, npad], bf16)
            asl = npad // 128 // atom_slices * 128
            for s0 in range(0, npad, asl):
                s1 = min(s0 + asl, npad)
                nc.sync.dma_start(atomT_sb[:, s0:s1], atomT[:, s0:s1])
            w_sb = const.tile([128, 256], bf16)
            nc.sync.dma_start(w_sb[:], wswdT[:])
            b_sb = const.tile([1, 256], bf16)
            nc.sync.dma_start(b_sb[:], bias2[:])
            w1b_sb = const.tile([128, 128], bf16)
            nc.sync.dma_start(w1b_sb[:], w1bbc[:])
            sidx_sb = const.tile([128, ecp // 16], i16)
            nc.sync.dma_start(sidx_sb[:], sidx[:])
            didx_sb = const.tile([128, ecp // 16], i16)
            nc.sync.dma_start(didx_sb[:], didx[:])
            bond_sb = const.tile([128, ecp // 128], f32)
            nc.sync.dma_start(bond_sb[:], bond[:])
            ones_sb = const.tile([1, 128], bf16)
            nc.vector.memset(ones_sb[:], 1.0)

            # ---- Phase 1: AsAd[n, 0:128]=As, [128:256]=Ad (+bias on As half)
            # 4 node-chunks per PSUM tile: 79 serial PE->ACT->DMA chains
            # become 20, cutting ~600ns of dependency latency per hop.
            P1B = p1_batch
            nchunk = npad // 128
            for i0 in range(0, nchunk, P1B):
                nb = min(P1B, nchunk - i0)
                ps = psum.tile([128, P1B, 256], f32, tag="ps")
                for j in range(nb):
                    i = i0 + j
                    nc.tensor.matmul(
                        ps[:, j, :],
                        atomT_sb[:, i * 128 : (i + 1) * 128],
                        w_sb[:],
                        start=True,
                        stop=False,
                    )
                    nc.tensor.matmul(
                        ps[:, j, :], ones_sb[:], b_sb[:], start=False, stop=True
                    )
                ab = p1.tile([128, P1B, 256], bf16, tag="ab")
                nc.scalar.copy(ab[:, :nb, :], ps[:, :nb, :])
                nc.sync.dma_start(
                    asad[i0 * 128 : (i0 + nb) * 128, :].rearrange(
                        "(c p) f -> p c f", c=nb
                    ),
                    ab[:, :nb, :],
                )

            tc.strict_bb_all_engine_barrier()

            # ---- Phase 2: per-edge gather + fused elementwise
            # (repeat>1 re-runs phase 2 in a device-side loop writing
            # identical values — used only by the wall-clock differencing
            # timer. The body never references the induction variable.)
            loop_cm = (
                tc.For_i(0, repeat, 1)
                if repeat > 1
                else contextlib.nullcontext()
            )
            with loop_cm:
                phase2(
                    nc, tc, nt, tile_e, nblk, s16, gather_chunk, single_packet,
                    gather_queues, g, acc, f32, asad, sidx_sb, didx_sb,
                    bond_sb, w1b_sb, out, mult, add, amax, variant,
                )
    nc.compile()
    return nc


def phase2(
    nc, tc, nt, tile_e, nblk, s16, gather_chunk, single_packet, gather_queues,
    g, acc, f32, asad, sidx_sb, didx_sb, bond_sb, w1b_sb, out, mult, add, amax,
    variant=0,
):
    bf16 = mybir.dt.bfloat16
    ck = min(gather_chunk or tile_e, tile_e)
    assert tile_e % ck == 0 and ck % 128 == 0
    # One shared register for the (constant) gather index count — a fresh
    # to_reg per gather costs a Pool-SEQ RegisterMove (~1us each, ~80/iter).
    nidx_reg = nc.gpsimd.to_reg(ck)
    for t in range(nt):
        ts_ = g.tile([128, nblk, 128], bf16, tag="ts")
        td_ = g.tile([128, nblk, 128], bf16, tag="td")
        gq = 0
        if variant == 1:
            nc.vector.memset(ts_[:], 0.25)
        if variant in (1, 2):
            nc.vector.memset(td_[:], 0.25)
        for c0 in range(0, tile_e, ck) if variant != 1 else []:
            # chunk c0..c0+ck of slot space = output blocks
            # c0//128..(c0+ck)//128, idx columns c0//16..(c0+ck)//16
            for tile_, idxs_sb, col0 in (
                (ts_, sidx_sb, 0),
                (td_, didx_sb, 128),
            )[: (1 if variant == 2 else 2)]:
                nc.gpsimd.dma_gather(
                    tile_[:, c0 // 128 : (c0 + ck) // 128, :],
                    asad[:, col0 : col0 + 128],
                    idxs_sb[
                        :, t * s16 + c0 // 16 : t * s16 + (c0 + ck) // 16
                    ],
                    ck,
                    nidx_reg,
                    128,
                    elem_step=256,
                    single_packet=single_packet,
                    queue_num=(t * (tile_e // ck) * 2 + gq) % gather_queues,
                )
                gq += 1
        pre = acc.tile([128, nblk, 128], bf16, tag="pre")
        if variant in (5, 6):
            # bond term in one broadcast DVE op: bt[p,b,f] = w1b[f]*bond[p,b]
            bt = acc.tile([128, nblk, 128], bf16, tag="bt")
            nc.vector.tensor_tensor(
                bt[:],
                w1b_sb[:].rearrange("p (b f) -> p b f", b=1).to_broadcast(
                    [128, nblk, 128]
                ),
                bond_sb[:, t * nblk : (t + 1) * nblk].to_broadcast(
                    [128, nblk, 128]
                ),
                mult,
            )
            nc.vector.tensor_add(pre[:], ts_[:], td_[:])
            nc.vector.tensor_add(pre[:], pre[:], bt[:])
            ob = acc.tile([128, nblk, 128], bf16, tag="ob")
            if variant == 5:
                nc.scalar.activation(
                    ob[:], pre[:], mybir.ActivationFunctionType.Lrelu,
                    alpha=NEG_SLOPE,
                )
            else:
                nc.vector.scalar_tensor_tensor(
                    ob[:], pre[:], NEG_SLOPE, pre[:], op0=mult, op1=amax
                )
            nc.sync.dma_start(
                out[t, :, :], ob[:, :, :].rearrange("p b f -> p (b f)")
            )
            continue
        nc.vector.tensor_add(pre[:], ts_[:], td_[:])
        if variant in (3, 4):
            # bond term on the (otherwise idle) scalar engine:
            # bt[:, b, :] = Copy(w1b * bond_scalar), then one DVE add.
            bt = acc.tile([128, nblk, 128], bf16, tag="bt")
            for b in range(nblk):
                nc.scalar.activation(
                    bt[:, b, :],
                    w1b_sb[:],
                    mybir.ActivationFunctionType.Copy,
                    scale=bond_sb[:, t * nblk + b : t * nblk + b + 1],
                )
            nc.vector.tensor_add(pre[:], pre[:], bt[:])
        else:
            for b in range(nblk):
                # pre[:, b, :] += w1b * bond  (bond scalar per partition)
                nc.vector.scalar_tensor_tensor(
                    pre[:, b, :],
                    w1b_sb[:],
                    bond_sb[:, t * nblk + b : t * nblk + b + 1],
                    pre[:, b, :],
                    op0=mult,
                    op1=add,
                )
        ob = acc.tile([128, nblk, 128], bf16, tag="ob")
        # leaky_relu: max(x, 0.01*x)
        if variant == 4:
            nc.scalar.activation(
                ob[:], pre[:], mybir.ActivationFunctionType.Lrelu,
                alpha=NEG_SLOPE,
            )
        else:
            nc.vector.scalar_tensor_tensor(
                ob[:], pre[:], NEG_SLOPE, pre[:], op0=mult, op1=amax
            )
        nc.sync.dma_start(out[t, :, :], ob[:, :, :].rearrange("p b f -> p (b f)"))


def _get_program():
    global _PROGRAM
    if _PROGRAM is None:
        _PROGRAM = _build_program()
    return _PROGRAM


def _wrap_idx(vals, tile_e=TILE_E, nt=NT):
    """[ecp] int array -> [128, ecp//16] int16 in dma_gather layout.

    Slot i of tile t gathers the row for edge t*tile_e + (i%128)*nblk +
    (i//128) (so the output tile DMAs contiguously per partition), and
    slot i's index lives at partition i%16 (replicated x8), column i//16.
    """
    nblk = tile_e // 128
    i = np.arange(tile_e)
    perm = (i % 128) * nblk + (i // 128)
    lst = vals.reshape(nt, tile_e)[:, perm]            # slot order per tile
    w = lst.reshape(nt, tile_e // 16, 16).transpose(0, 2, 1)  # [nt, 16, s16]
    w = np.tile(w, (1, 8, 1))                          # replicate to 128 parts
    return np.ascontiguousarray(
        w.transpose(1, 0, 2).reshape(128, -1)
    ).astype(np.int16)


# Reorder each core's edges on the host (free; kernel() un-sorts the
# output) so the gathers read table rows with locality instead of random
# 512B rows. Mode 1: sort by dst (dst gather becomes runs of adjacent
# rows). Mode 2: serpentine 2D bucketing by (src-block, dst-block) —
# both gather streams stay within ~512-row windows per tile.
SORT_MODE = int(os.environ.get("KERNEL_SORT_MODE", "2"))
_LAST_PERMS = None


def _edge_order(src_c, dst_c):
    if SORT_MODE == 1:
        return np.argsort(dst_c, kind="stable")
    if SORT_MODE == 2:
        B = 512
        bs = src_c // B
        bd = dst_c // B
        nbd = (N + B - 1) // B
        key = bs * nbd + np.where(bs % 2 == 0, bd, nbd - 1 - bd)
        return np.lexsort((dst_c, key))
    return np.arange(len(src_c))


def _host_prep(inputs):
    global _LAST_PERMS
    atom = np.ascontiguousarray(np.asarray(inputs["atom_feats"], dtype=np.float32))
    bondf = np.asarray(inputs["bond_feats"], dtype=np.float32).reshape(-1)
    src = np.asarray(inputs["src"]).astype(np.int64)
    dst = np.asarray(inputs["dst"]).astype(np.int64)
    perms = []
    if SORT_MODE:
        src, dst, bondf = src.copy(), dst.copy(), bondf.copy()
        for c in range(N_CORES):
            sl = slice(c * EC, (c + 1) * EC)
            p = _edge_order(src[sl], dst[sl])
            src[sl], dst[sl], bondf[sl] = src[sl][p], dst[sl][p], bondf[sl][p]
            perms.append(p)
    _LAST_PERMS = perms
    W1 = np.asarray(inputs["W1"], dtype=np.float32)
    b1 = np.asarray(inputs["b1"], dtype=np.float32)
    W2 = np.asarray(inputs["W2"], dtype=np.float32)
    b2 = np.asarray(inputs["b2"], dtype=np.float32)

    Ws = W1[:, :D] + W2                     # [128, 128]
    Wd = W1[:, D : 2 * D] + W2              # [128, 128]
    w1b = W1[:, 2 * D]                      # [128]
    bias = b1 + b2                          # [128]

    atomT = np.zeros((128, NPAD), BF16)
    atomT[:, :N] = atom.T.astype(BF16)
    wswdT = np.ascontiguousarray(
        np.concatenate([Ws.T, Wd.T], axis=1).astype(BF16)
    )                                       # [128(fin), 256]
    bias2 = np.concatenate([bias, np.zeros(D, np.float32)])[None, :]
    bias2 = np.ascontiguousarray(bias2.astype(BF16))
    w1bbc = np.ascontiguousarray(np.tile(w1b[None, :], (128, 1)).astype(BF16))

    in_maps = []
    for c in range(N_CORES):
        sl = slice(c * EC, (c + 1) * EC)
        sp = np.zeros(ECP, np.int64)
        sp[:EC] = src[sl]
        dp = np.zeros(ECP, np.int64)
        dp[:EC] = dst[sl]
        bp = np.zeros(ECP, np.float32)
        bp[:EC] = bondf[sl]
        # bond[p, t*NBLK+b] = bp[t*TILE_E + p*NBLK + b]
        bperm = np.ascontiguousarray(
            bp.reshape(NT, 128, NBLK).transpose(1, 0, 2).reshape(128, NT * NBLK)
        )
        in_maps.append(
            {
                "atomT": atomT,
                "wswdT": wswdT,
                "bias2": bias2,
                "w1bbc": w1bbc,
                "sidx": _wrap_idx(sp),
                "didx": _wrap_idx(dp),
                "bond": bperm,
            }
        )
    return in_maps


def kernel(**inputs) -> np.ndarray:
    global LAST_EXEC_NS, LAST_RESULTS
    in_maps = _host_prep(inputs)
    nc = _get_program()
    res = run_bass_kernel_spmd(
        nc, in_maps, list(range(N_CORES)), trace=KERNEL_TRACE
    )
    LAST_EXEC_NS = res.exec_time_ns
    LAST_RESULTS = res
    outs = []
    for c in range(N_CORES):
        o = np.asarray(res.results[c]["out"]).reshape(ECP, 128)[:EC]
        if SORT_MODE:
            inv = np.empty(EC, np.int64)
            inv[_LAST_PERMS[c]] = np.arange(EC)
            o = o[inv]
        outs.append(o)
    return np.concatenate(outs, axis=0).astype(np.float32)

